# revision 19
# baseline (speedup 1.0000x reference)
"""Trainium2 Bass kernel: single-head attention with query-axis softmax.

Reference semantics (per batch element b):
    q = X @ Wq.T + bq ; k = X @ Wk.T + bk ; v = X @ Wv.T + bv          # [T,E]
    s = q @ k.T / sqrt(E), s[i,j] = -inf for j > i (strict upper tri)
    attn = softmax(s, axis=-2)          # over the QUERY axis i (faithful)
    out = attn @ v                      # [T,E]

Sharding: data-parallel over batch. B=8 batch elements -> one NeuronCore
each; host transposes/casts inputs, device computes, host stacks outputs.

Device strategy (per core): the q/k projections are folded algebraically
into a single matmul G = (Wq^T Wk) X^T computed feature-major [E,T] (the
host precomputes A = Wq^T Wk and the rank-1 bias terms); v is token-major
[T,E]. Scores are built transposed, sT[j,i] = k_j . q_i = G_j . X_i + bias,
so the softmax reduction (over i) runs along the free axis. exp rows are
normalized implicitly by folding 1/colsum[j] into v[j,:]. The causal
structure (only i >= j is live) halves the score and attn@v matmul work.
Projections and attn@v run in fp16 (fp32 accumulation in PSUM); the
score contraction (phase 2) runs in fp8e4 with DoubleRow perf mode
(2 contraction rows per PE cell -> ~1.8x fewer PE cycles), with G and
X quantized at 16x scale and the 256x product prescale folded into the
exp activation scale. The normalized v rows are pre-scaled by 1024 to
stay clear of fp16 subnormals; the epilogue scales back and stores the
output fp16 (host upcasts to fp32).
"""

import math
from contextlib import ExitStack

import numpy as np

import concourse.bacc as bacc
import concourse.tile as tile
from concourse import mybir
from concourse.bass_utils import run_bass_kernel_spmd
from concourse.tile import add_dep_helper

F16 = np.float16

P = 128          # partitions
T = 2048         # tokens
E = 1024         # embed
B = 8            # batch == n_cores
EO = E // P      # 8 contraction tiles
FO = E // P      # 8 feature tiles
NT = T // P      # 16 token tiles
TCH = T // 512   # 4 chunks of 512 tokens
SCALE = 1.0 / math.sqrt(E)   # 1/32
NEG = -1.0e30
VSC = 1024.0   # v' pre-scale keeping fp16 out of subnormal range

_CACHE = {}


def _build_nc():
    f32 = mybir.dt.float32
    f16 = mybir.dt.float16
    Act = mybir.ActivationFunctionType

    nc = bacc.Bacc(None, target_bir_lowering=False)

    f8 = mybir.dt.float8e4

    # chunk-major X^T: [tci, p, eo, t'] = X[tci*512+t', eo*128+p], so each
    # 512-token chunk is one contiguous 1MB DMA
    XT = nc.dram_tensor("XT", [TCH, P, EO, 512], f16, kind="ExternalInput")
    # same layout, fp8e4 at 16x scale: phase-2 moving operand (DoubleRow)
    XT8 = nc.dram_tensor("XT8", [TCH, P, EO, 512], f8, kind="ExternalInput")
    # A = Wq^T @ Wk blocked as [fo, p, eo, c] = A[fo*128+c, eo*128+p]
    AQ = nc.dram_tensor("AQ", [FO, P, EO, P], f16, kind="ExternalInput")
    # [p, eo, f] = Wv.T[eo*128+p, f]  (moving operand, f contiguous)
    WV = nc.dram_tensor("WV", [P, EO, E], f16, kind="ExternalInput")
    U2 = nc.dram_tensor("U2", [P, FO], f32, kind="ExternalInput")   # 16*Wq^T bk
    BX2 = nc.dram_tensor("BX2", [P, NT], f32, kind="ExternalInput")  # (X.w+c0)/32
    BV = nc.dram_tensor("BV", [P, E], f16, kind="ExternalInput")   # bv row bcast
    MSK = nc.dram_tensor("MSK", [P, P], f32, kind="ExternalInput")  # 0 / -1e30
    OUT = nc.dram_tensor("OUT", [T, E], f16, kind="ExternalOutput")

    with tile.TileContext(nc) as tc, ExitStack() as ctx:
        persist = ctx.enter_context(tc.tile_pool(name="persist", bufs=1))
        small = ctx.enter_context(tc.tile_pool(name="small", bufs=1))
        outst = ctx.enter_context(tc.tile_pool(name="outst", bufs=2))
        ps = ctx.enter_context(tc.tile_pool(name="ps", bufs=5, space="PSUM"))
        ps3 = ctx.enter_context(tc.tile_pool(name="ps3", bufs=3, space="PSUM"))

        # chunk 0 is split into eo-halves on two DMA queues so the first
        # matmul group (eo 0-3) can start after only 512KB has landed
        xt0h = [
            persist.tile([P, EO // 2, 512], f16, tag=f"xt0{h}", name=f"xt0{h}")
            for h in range(2)
        ]
        xtc = [None] + [
            persist.tile([P, EO, 512], f16, tag=f"xt{i}", name=f"xt{i}")
            for i in range(1, TCH)
        ]                                      # 32 KB/part total
        xt8c = [
            persist.tile([P, EO, 512], f8, tag=f"x8{i}", name=f"x8{i}")
            for i in range(TCH)
        ]                                      # 16 KB/part total
        gt = persist.tile([P, FO, T], f8)     # 16   G = 16*(A X^T + u), fp8
        v = persist.tile([P, NT, E], f16)     # 32
        wv = persist.tile([P, EO, E], f16)    # 16

        def xslice(tci, eo, c0, c1):
            """fp16 X^T operand [128, c1-c0] for chunk tci, contraction blk eo."""
            if tci == 0:
                return xt0h[eo // 4][:, eo % 4, c0:c1]
            return xtc[tci][:, eo, c0:c1]

        # unnormalized exp(scores^T) rows, all key tiles packed in one tile
        # (exact widths, ~34KB/part); eoff[jt] = start col of key tile jt
        eoff = [0] * NT
        for jt in range(1, NT):
            eoff[jt] = eoff[jt - 1] + (T - (jt - 1) * P)
        etp = persist.tile([P, eoff[NT - 1] + (T - (NT - 1) * P)], f16)
        sums = persist.tile([P, NT, TCH], f32)
        rcol = persist.tile([P, NT, 1], f32)

        u2_sb = small.tile([P, FO], f32)
        bx2_sb = small.tile([P, NT], f32)
        bv_sb = small.tile([P, E], f16)
        msk_sb = small.tile([P, P], f32)

        aqs = [
            persist.tile([P, EO, P], f16, tag=f"aq{fo}", name=f"aq{fo}")
            for fo in range(FO)
        ]
        # startup DMA plan, three queues in parallel:
        #   sync:   xt0 lo-half, aq1-7, xt chunks 2-3
        #   gpsimd: xt0 hi-half, xt chunk 1 (then deferred wv + fp8 X)
        #   scalar: aq0, u2, then the small constants
        # so the first matmul group waits only on 512KB + 256KB.
        xt_dmas = [nc.sync.dma_start(xt0h[0][:], XT[0, :, 0:EO // 2])]
        nc.gpsimd.dma_start(xt0h[1][:], XT[0, :, EO // 2:EO])
        nc.scalar.dma_start(aqs[0][:], AQ[0])
        nc.scalar.dma_start(u2_sb[:], U2[:])
        xt_dmas.append(nc.gpsimd.dma_start(xtc[1][:], XT[1]))
        for fo in range(1, FO):
            nc.sync.dma_start(aqs[fo][:], AQ[fo])
        xt_dmas += [
            nc.sync.dma_start(xtc[ci][:], XT[ci]) for ci in range(2, TCH)
        ]
        nc.scalar.dma_start(bx2_sb[:], BX2[:])
        nc.scalar.dma_start(bv_sb[:], BV[:])
        nc.scalar.dma_start(msk_sb[:], MSK[:])

        # PE warm-up during the initial DMA wait: ~4.3us of dummy matmuls on
        # zeroed SBUF clears the HAM clock throttle (K=4/8 -> 8/8) before the
        # first real matmul group, which would otherwise run at 1.2 GHz.
        warm = small.tile([P, 512], f16)
        nc.vector.memset(warm[:], 0.0)
        for _ in range(24):
            pw = ps3.tile([P, 256], mybir.dt.float32, tag="ps3")
            nc.tensor.matmul(
                pw[:], warm[:, 0:P], warm[:, 0:256], start=True, stop=True
            )

        # ---- Phase 1a: G = A X^T + u (feature-major) ------------------------
        # G[e1, t] = sum_e2 A[e1, e2] X.T[e2, t] + u[e1]; the q and k
        # projections never materialize -- scores contract G against X^T.
        # tci-outer so the first 8 groups need only X chunk 0 (1MB), letting
        # compute start while the rest of X streams in.
        for tci in range(TCH):
            for fo in range(FO):
                pt = ps.tile([P, 512], mybir.dt.float32, tag="ps")
                for eo in range(EO):
                    nc.tensor.matmul(
                        pt[:],
                        aqs[fo][:, eo, :],
                        xslice(tci, eo, 0, 512),
                        start=(eo == 0),
                        stop=(eo == EO - 1),
                    )
                # 16*(psum + u) -> fp8 SBUF (ScalarE); U2 is host-premultiplied
                # by 16 so bias matches the 16x quantization scale
                nc.scalar.activation(
                    out=gt[:, fo, tci * 512:(tci + 1) * 512],
                    in_=pt[:],
                    func=Act.Identity,
                    scale=16.0,
                    bias=u2_sb[:, fo:fo + 1],
                )

        # ---- Phase 1b: v projection (token-major) ---------------------------
        # v[t, f] = sum_e X.T[e, t] * Wv.T[e, f] + bv[f]
        wv_dma = nc.gpsimd.dma_start(wv[:], WV[:])
        # keep the 2MB wv transfer out of the startup-critical DMA window
        add_dep_helper(
            wv_dma.ins, xt_dmas[-1].ins, reason="defer wv load past xt stream"
        )
        # fp8 X copy for phase 2 (needed ~120us in) follows wv on gpsimd
        for ci in range(TCH):
            nc.gpsimd.dma_start(xt8c[ci][:], XT8[ci])
        for to in range(NT):
            for half in range(2):
                pt = ps.tile([P, 512], mybir.dt.float32, tag="ps")
                for eo in range(EO):
                    nc.tensor.matmul(
                        pt[:],
                        xslice(to // 4, eo, (to % 4) * P, (to % 4 + 1) * P),
                        wv[:, eo, half * 512:(half + 1) * 512],
                        start=(eo == 0),
                        stop=(eo == EO - 1),
                    )
                # psum + bv (free-axis broadcast row, pre-materialized) -> fp16
                nc.vector.tensor_add(
                    out=v[:, to, half * 512:(half + 1) * 512],
                    in0=pt[:],
                    in1=bv_sb[:, half * 512:(half + 1) * 512],
                )

        # ---- Phase 2+3 interleaved over key/query tiles ---------------------
        for jt in range(NT):
            n_i = T - jt * P          # live columns i >= jt*128
            ci0 = jt // 4             # first 512-aligned x chunk with live cols
            nch = TCH - ci0
            for c in range(ci0, TCH):
                lo = (jt % 4) * P if c == ci0 else 0
                w = 512 - lo
                ecol = c * 512 + lo - jt * P   # dest col within et[jt]
                pt = ps.tile([P, 512], mybir.dt.float32, tag="ps")
                # fp8 DoubleRow: 4 matmuls over 256-deep contraction pairs;
                # psum accumulates 256 * s_core (16x on each operand)
                for fo2 in range(FO // 2):
                    nc.tensor.matmul(
                        pt[:, :w],
                        gt[:, 2 * fo2:2 * fo2 + 2, jt * P:(jt + 1) * P],
                        xt8c[c][:, 2 * fo2:2 * fo2 + 2, lo:512],
                        start=(fo2 == 0),
                        stop=(fo2 == FO // 2 - 1),
                        perf_mode=mybir.MatmulPerfMode.DoubleRow,
                    )
                if c == ci0:
                    # causal mask on the diagonal 128x128 block (additive -1e30)
                    nc.vector.tensor_add(
                        out=pt[:, :P], in0=pt[:, :P], in1=msk_sb[:]
                    )
                # exp((s_core - 1e30*mask)/32 + (x.w + bq.bk)/32) with row sum;
                # scale folds away the 256x operand prescale
                nc.scalar.activation(
                    out=etp[:, eoff[jt] + ecol:eoff[jt] + ecol + w],
                    in_=pt[:, :w],
                    func=Act.Exp,
                    scale=SCALE / 256.0,
                    bias=bx2_sb[:, jt:jt + 1],
                    accum_out=sums[:, jt, c - ci0:c - ci0 + 1],
                )
            # softmax denominator for this key tile; fold 1/colsum into v
            nc.vector.tensor_reduce(
                out=rcol[:, jt, :],
                in_=sums[:, jt, :nch],
                axis=mybir.AxisListType.X,
                op=mybir.AluOpType.add,
            )
            nc.vector.reciprocal(out=rcol[:, jt, :], in_=rcol[:, jt, :])
            nc.vector.tensor_scalar(
                out=v[:, jt, :],
                in0=v[:, jt, :],
                scalar1=rcol[:, jt, :],
                scalar2=float(VSC),
                op0=mybir.AluOpType.mult,
                op1=mybir.AluOpType.mult,
            )

            # Phase 3: out rows for query tile it == jt (needs et[0..jt], v'[0..jt])
            it = jt
            ob = outst.tile([P, E], f16, tag="ob")
            # last tile: 256-wide pieces so the final copy+DMA mostly hides
            # under the preceding piece's matmuls (shorter exposed tail)
            wo = 256 if it == NT - 1 else 512
            for piece in range(E // wo):
                c0 = piece * wo
                po = ps3.tile([P, wo], mybir.dt.float32, tag="ps3")
                for j2 in range(it + 1):
                    off = eoff[j2] + (it - j2) * P
                    nc.tensor.matmul(
                        po[:], etp[:, off:off + P],
                        v[:, j2, c0:c0 + wo],
                        start=(j2 == 0), stop=(j2 == it),
                    )
                if piece % 2 == 0:
                    nc.scalar.mul(out=ob[:, c0:c0 + wo], in_=po[:], mul=1.0 / VSC)
                    nc.scalar.dma_start(
                        OUT[it * P:(it + 1) * P, c0:c0 + wo], ob[:, c0:c0 + wo]
                    )
                else:
                    nc.vector.tensor_scalar_mul(
                        out=ob[:, c0:c0 + wo], in0=po[:], scalar1=1.0 / VSC
                    )
                    nc.sync.dma_start(
                        OUT[it * P:(it + 1) * P, c0:c0 + wo], ob[:, c0:c0 + wo]
                    )

    nc.compile()
    return nc


def _prep_inputs(X, Wq, bq, Wk, bk, Wv, bv):
    """Host-side prep: fold the q/k projections into A = Wq^T Wk (so the
    device does one E x E x T matmul instead of two), plus layouts/casts.

    s[i,j] = q_i . k_j = X_i A X_j^T + X_i.u + X_j.w + bq.bk
    with u = Wq^T bk (added per-partition to G = A X^T) and the per-batch
    xw = X.w + bq.bk folded into the exp bias (pre-divided by sqrt(E)).
    """
    X = np.asarray(X, dtype=np.float64)
    Wq = np.asarray(Wq, dtype=np.float64)
    Wk = np.asarray(Wk, dtype=np.float64)
    bq = np.asarray(bq, dtype=np.float64)
    bk = np.asarray(bk, dtype=np.float64)

    # XT[b, tci, p, eo, t'] = X[b, tci*512+t', eo*128+p]  (chunk-contiguous)
    xt4 = np.ascontiguousarray(
        X.transpose(0, 2, 1)
        .reshape(B, EO, P, TCH, 512)
        .transpose(0, 3, 2, 1, 4)
    ).astype(F16)

    A = Wq.T @ Wk                      # [E, E]
    # [fo, p, eo, c] = A[fo*128+c, eo*128+p]   (lhsT blocks for G = A X^T)
    aq4 = np.ascontiguousarray(
        A.reshape(FO, P, EO, P).transpose(0, 3, 2, 1)
    ).astype(F16)

    u = Wq.T @ bk                      # [E]
    # premultiplied by 16: G is stored fp8 at 16x scale on device
    u2 = np.ascontiguousarray((16.0 * u).reshape(FO, P).T.astype(np.float32))

    # fp8e4 copy of X^T at 16x scale (phase-2 DoubleRow moving operand)
    import ml_dtypes

    xt8 = (16.0 * np.asarray(xt4, dtype=np.float32)).astype(ml_dtypes.float8_e4m3)

    w_vec = Wk.T @ bq                  # [E]
    c0 = float(bq @ bk)
    # per-batch exp bias: (X[b].w + c0) / sqrt(E), laid out [p, jt]
    xw = (X @ w_vec + c0) * SCALE      # [B, T]
    bx2 = np.ascontiguousarray(
        xw.reshape(B, NT, P).transpose(0, 2, 1).astype(np.float32)
    )

    # WV: [p, eo, f] = Wv.T[eo*128+p, f]
    WvT = np.asarray(Wv, dtype=np.float32).T
    wv4 = np.ascontiguousarray(
        WvT.reshape(EO, P, E).transpose(1, 0, 2)
    ).astype(F16)

    bvr = np.ascontiguousarray(
        np.broadcast_to(np.asarray(bv, dtype=np.float32), (P, E))
    ).astype(F16)

    ii = np.arange(P)
    mask = np.where(ii[None, :] >= ii[:, None], 0.0, NEG).astype(np.float32)

    shared = {"AQ": aq4, "WV": wv4, "U2": u2, "BV": bvr, "MSK": mask}
    return [
        dict(
            shared,
            XT=np.ascontiguousarray(xt4[b]),
            XT8=np.ascontiguousarray(xt8[b]),
            BX2=np.ascontiguousarray(bx2[b]),
        )
        for b in range(B)
    ]


def _postprocess(out_stacked):
    """[B, T, E] stacked per-core OUT -> full fp32 output."""
    return np.ascontiguousarray(out_stacked.astype(np.float32))


def run_sharded(inputs, trace=False, **kwargs):
    """Build (cached), run on 8 cores, gather. Returns (out, BassKernelResults)."""
    if "nc" not in _CACHE:
        _CACHE["nc"] = _build_nc()
    nc = _CACHE["nc"]
    in_maps = _prep_inputs(**inputs)
    res = run_bass_kernel_spmd(
        nc, in_maps, core_ids=list(range(B)), trace=trace, **kwargs
    )
    out = np.stack([np.asarray(r["OUT"], dtype=np.float32) for r in res.results])
    return out, res


def kernel(**inputs) -> np.ndarray:
    out, _ = run_sharded(inputs)
    return out

